# revision 2
# baseline (speedup 1.0000x reference)
"""Trainium2 Bass kernel for nn_Net_210311 (gnn_message_passing).

Strategy (per sharding hint): data-parallel over the B=64 graph dimension,
8 graphs per NeuronCore, weights replicated.

Device algorithm per graph:
  - Message aggregation (segment_sum over edges) is cast as a dense per-graph
    adjacency matmul: aggT[f, d] = sum_s h[s, f] * A[s, d], with A the
    1024x1024 src->dst edge-count matrix (exact in fp32r: small integers).
  - All precision-critical matmuls run as split-2 float32r (x = x_hi + x_lo,
    both fp32r-rounded): 2 full-rate matmuls instead of one 4x-slow fp32
    matmul, exact to ~2^-24.
  - TopK pooling per graph: scores -> gpsimd kth_largest (exact order
    statistic via the desc[k_adj+1] output), threshold compare -> keep mask,
    gate = keep * tanh(score). nmask/emask multiplies of the reference are
    absorbed: gated h is zero at dropped nodes, and masked scores are driven
    to -1e30 before ranking.
  - Readouts: mean via PE ones-matvec (denominator = exact kept count),
    max via transpose + free-axis reduce with -1e30 penalty at masked nodes.
  - MLP head + log_softmax in plain fp32 on-device.
"""
import sys
import os

sys.path.insert(0, "/opt/trn_rl_repo")

import numpy as np

import concourse.bass as bass
import concourse.tile as tile
from concourse import bacc, mybir
from concourse.bass_utils import run_bass_kernel_spmd
from concourse.masks import make_identity

dt = mybir.dt
AF = mybir.ActivationFunctionType
ALU = mybir.AluOpType

B = 64          # graphs
NP = 1024       # nodes per graph
H = 128
NCORES = 8
GPC = B // NCORES  # graphs per core = 8
T = NP // 128      # node tiles per graph = 8

# pooling constants: (kept count, k for negated kth_largest)
# pool1: ceil(0.8*1024)=820 ; pool2: ceil(0.8*820)=656 ; pool3: ceil(0.8*656)=525
K_DENOM = [820, 656, 525]
K_NEG = [NP + 1 - k for k in K_DENOM]           # 205, 369, 500
Q_KTH = [1.0 - (kn - 1.5) / (NP - 1.0) for kn in K_NEG]

NEG_BIG = 1.0e30


def _build():
    nc = bacc.Bacc("TRN2", target_bir_lowering=False, debug=False)

    # ---------- DRAM I/O ----------
    adj_d = nc.dram_tensor("adj", [GPC * NP, NP], dt.float32r, kind="ExternalInput")
    xt_hi_d = nc.dram_tensor("xt_hi", [128, GPC * T, 4], dt.float32r, kind="ExternalInput")
    xt_lo_d = nc.dram_tensor("xt_lo", [128, GPC * T, 4], dt.float32r, kind="ExternalInput")
    xT_hi_d = nc.dram_tensor("xT_hi", [4, GPC, NP], dt.float32r, kind="ExternalInput")
    xT_lo_d = nc.dram_tensor("xT_lo", [4, GPC, NP], dt.float32r, kind="ExternalInput")
    w1r_d = nc.dram_tensor("w1r", [4, H], dt.float32, kind="ExternalInput")
    w1l_d = nc.dram_tensor("w1l", [4, H], dt.float32, kind="ExternalInput")
    w2r_d = nc.dram_tensor("w2r", [H, H], dt.float32, kind="ExternalInput")
    w2l_d = nc.dram_tensor("w2l", [H, H], dt.float32, kind="ExternalInput")
    w3r_d = nc.dram_tensor("w3r", [H, H], dt.float32, kind="ExternalInput")
    w3l_d = nc.dram_tensor("w3l", [H, H], dt.float32, kind="ExternalInput")
    b1_d = nc.dram_tensor("b1c", [H, 1], dt.float32, kind="ExternalInput")
    b2_d = nc.dram_tensor("b2c", [H, 1], dt.float32, kind="ExternalInput")
    b3_d = nc.dram_tensor("b3c", [H, 1], dt.float32, kind="ExternalInput")
    p1_d = nc.dram_tensor("p1", [H, 1], dt.float32, kind="ExternalInput")
    p2_d = nc.dram_tensor("p2", [H, 1], dt.float32, kind="ExternalInput")
    wl1_d = nc.dram_tensor("wl1", [2 * H, H], dt.float32, kind="ExternalInput")
    bl1_d = nc.dram_tensor("bl1c", [H, 1], dt.float32, kind="ExternalInput")
    wl2_d = nc.dram_tensor("wl2", [H, 64], dt.float32, kind="ExternalInput")
    bl2_d = nc.dram_tensor("bl2c", [64, 1], dt.float32, kind="ExternalInput")
    wl3_d = nc.dram_tensor("wl3", [64, 7], dt.float32, kind="ExternalInput")
    bl3_d = nc.dram_tensor("bl3r", [1, 7], dt.float32, kind="ExternalInput")
    out_d = nc.dram_tensor("out", [GPC, 7], dt.float32, kind="ExternalOutput")

    with tile.TileContext(nc) as tc:
        with tc.tile_pool(name="pers", bufs=1) as pers, \
             tc.tile_pool(name="graph", bufs=2) as gpool, \
             tc.tile_pool(name="st1", bufs=1) as st1, \
             tc.tile_pool(name="st2", bufs=2) as st2, \
             tc.tile_pool(name="xtg", bufs=2) as xtg, \
             tc.tile_pool(name="small", bufs=2) as sm, \
             tc.tile_pool(name="ps_acc", bufs=2, space="PSUM") as ps_acc, \
             tc.tile_pool(name="ps_tp", bufs=2, space="PSUM") as ps_tp:

            # ---------- setup: load weights ----------
            def load(dram, shape, dtype=dt.float32):
                t_ = pers.tile(shape, dtype, tag=f"ld_{dram.name}")
                nc.sync.dma_start(t_[:], dram[:])
                return t_

            w1r_f = load(w1r_d, [4, H])
            w1l_f = load(w1l_d, [4, H])
            w2r_f = load(w2r_d, [H, H])
            w2l_f = load(w2l_d, [H, H])
            w3r_f = load(w3r_d, [H, H])
            w3l_f = load(w3l_d, [H, H])
            b1c = load(b1_d, [H, 1])
            b2c = load(b2_d, [H, 1])
            b3c = load(b3_d, [H, 1])
            p1_f = load(p1_d, [H, 1])
            p2_f = load(p2_d, [H, 1])
            wl1a = pers.tile([H, H], dt.float32, tag="wl1a")
            nc.sync.dma_start(wl1a[:], wl1_d[0:H, :])
            wl1b = pers.tile([H, H], dt.float32, tag="wl1b")
            nc.sync.dma_start(wl1b[:], wl1_d[H:2 * H, :])
            bl1c = load(bl1_d, [H, 1])
            wl2_f = load(wl2_d, [H, 64])
            bl2c = load(bl2_d, [64, 1])
            wl3_f = load(wl3_d, [64, 7])
            bl3r = load(bl3_d, [1, 7])
            xt_hi = load(xt_hi_d, [128, GPC * T, 4], dt.float32r)
            xt_lo = load(xt_lo_d, [128, GPC * T, 4], dt.float32r)

            ident = pers.tile([128, 128], dt.float32, tag="ident")
            make_identity(nc, ident[:])
            ones_col = pers.tile([128, 1], dt.float32, tag="ones_col")
            nc.vector.memset(ones_col[:], 1.0)
            ones8 = pers.tile([1, GPC], dt.float32, tag="ones8")
            nc.vector.memset(ones8[:], 1.0)

            # split-2 fp32r weights
            def split(f32_tile, shape, name):
                hi = pers.tile(shape, dt.float32r, tag=f"{name}_hi")
                nc.vector.tensor_copy(hi[:], f32_tile[:])
                lo = pers.tile(shape, dt.float32r, tag=f"{name}_lo")
                nc.vector.tensor_tensor(out=lo[:], in0=f32_tile[:],
                                        in1=hi[:].bitcast(dt.float32),
                                        op=ALU.subtract)
                return hi, lo

            w1r_hi, w1r_lo = split(w1r_f, [4, H], "w1r")
            w1l_hi, w1l_lo = split(w1l_f, [4, H], "w1l")
            w2r_hi, w2r_lo = split(w2r_f, [H, H], "w2r")
            w2l_hi, w2l_lo = split(w2l_f, [H, H], "w2l")
            w3r_hi, w3r_lo = split(w3r_f, [H, H], "w3r")
            w3l_hi, w3l_lo = split(w3l_f, [H, H], "w3l")

            # normalize pooling weights: p / ||p||
            def pnorm(p_f, name):
                sq = pers.tile([H, 1], dt.float32, tag=f"{name}_sq")
                nc.vector.tensor_tensor(sq[:], p_f[:], p_f[:], op=ALU.mult)
                ssum = ps_tp.tile([1, 1], dt.float32, tag="tp")
                nc.tensor.matmul(out=ssum[:], lhsT=sq[:], rhs=ones_col[:],
                                 start=True, stop=True)
                rt = pers.tile([1, 1], dt.float32, tag=f"{name}_rt")
                nc.scalar.activation(rt[:], ssum[:], AF.Sqrt)
                rec = pers.tile([1, 1], dt.float32, tag=f"{name}_rec")
                nc.vector.reciprocal(rec[:], rt[:])
                rbc = pers.tile([H, 1], dt.float32, tag=f"{name}_rbc")
                nc.gpsimd.partition_broadcast(rbc[:], rec[:])
                pn = pers.tile([H, 1], dt.float32, tag=f"{name}_n")
                nc.vector.tensor_tensor(pn[:], p_f[:], rbc[:], op=ALU.mult)
                return pn

            p1n = pnorm(p1_f, "p1")
            p2n = pnorm(p2_f, "p2")

            maxsum = pers.tile([H, GPC], dt.float32, tag="maxsum")
            nc.vector.memset(maxsum[:], 0.0)
            meansum = pers.tile([H, GPC], dt.float32, tag="meansum")
            nc.vector.memset(meansum[:], 0.0)

            layer_w = [
                (w1r_hi, w1r_lo, w1l_hi, w1l_lo, b1c, p1n),
                (w2r_hi, w2r_lo, w2l_hi, w2l_lo, b2c, p2n),
                (w3r_hi, w3r_lo, w3l_hi, w3l_lo, b3c, p2n),
            ]

            # ---------- per-graph pipeline ----------
            for g in range(GPC):
                a_sb = gpool.tile([128, T, NP], dt.float32r, tag="a_sb")
                for t in range(T):
                    nc.sync.dma_start(
                        a_sb[:, t, :],
                        adj_d[g * NP + t * 128:g * NP + (t + 1) * 128, :])
                xTg_hi = xtg.tile([4, NP], dt.float32r, tag="xTg_hi")
                nc.sync.dma_start(xTg_hi[:], xT_hi_d[:, g, :])
                xTg_lo = xtg.tile([4, NP], dt.float32r, tag="xTg_lo")
                nc.sync.dma_start(xTg_lo[:], xT_lo_d[:, g, :])

                nmask = sm.tile([128, T], dt.float32, tag="nmask0")
                nc.vector.memset(nmask[:], 1.0)
                ninf_pen = sm.tile([128, T], dt.float32, tag="ninf0")
                nc.vector.memset(ninf_pen[:], 0.0)

                h_hi = h_lo = hT_hi = hT_lo = None

                for l in range(3):
                    wr_hi, wr_lo, wl_hi, wl_lo, b_l, p_l = layer_w[l]
                    Ml = 4 if l == 0 else H

                    # aggT[f, d] = sum_s h[s, f] * A[s, d]  (split-2 fp32r)
                    aggT = ps_acc.tile([128, T, 128], dt.float32, tag="acc")
                    for nch in range(2):
                        dsl = slice(nch * 512, (nch + 1) * 512)
                        osl = slice(nch * 4, (nch + 1) * 4)
                        n_mm = 0
                        for t in range(T):
                            if l == 0:
                                lhs_pair = (xt_hi[:, g * T + t, :],
                                            xt_lo[:, g * T + t, :])
                            else:
                                lhs_pair = (h_hi[:, t, :], h_lo[:, t, :])
                            for lhsT in lhs_pair:
                                nc.tensor.matmul(
                                    out=aggT[:Ml, osl, :],
                                    lhsT=lhsT,
                                    rhs=a_sb[:, t, dsl],
                                    start=(n_mm == 0), stop=(n_mm == 2 * T - 1))
                                n_mm += 1

                    agg_hi = st1.tile([128, T, 128], dt.float32r, tag="agg_hi")
                    nc.scalar.copy(agg_hi[:Ml], aggT[:Ml])
                    agg_lo = st1.tile([128, T, 128], dt.float32r, tag="agg_lo")
                    nc.vector.tensor_tensor(out=agg_lo[:Ml], in0=aggT[:Ml],
                                            in1=agg_hi[:Ml].bitcast(dt.float32),
                                            op=ALU.subtract)

                    # convT[fo, d] = Wr.T @ aggT + Wl.T @ hT(prev)   (+bias+relu)
                    convT = ps_acc.tile([128, T, 128], dt.float32, tag="acc")
                    for nch in range(2):
                        osl = slice(nch * 4, (nch + 1) * 4)
                        if l == 0:
                            rL_hi = xTg_hi[:, nch * 512:(nch + 1) * 512]
                            rL_lo = xTg_lo[:, nch * 512:(nch + 1) * 512]
                        else:
                            rL_hi = hT_hi[:, osl, :]
                            rL_lo = hT_lo[:, osl, :]
                        mms = [
                            (wr_hi[:], agg_hi[:Ml, osl, :]),
                            (wr_hi[:], agg_lo[:Ml, osl, :]),
                            (wr_lo[:], agg_hi[:Ml, osl, :]),
                            (wl_hi[:], rL_hi),
                            (wl_hi[:], rL_lo),
                            (wl_lo[:], rL_hi),
                        ]
                        for i, (lh, rh) in enumerate(mms):
                            nc.tensor.matmul(out=convT[:, osl, :], lhsT=lh,
                                             rhs=rh,
                                             start=(i == 0), stop=(i == len(mms) - 1))

                    hT_relu = st1.tile([128, T, 128], dt.float32, tag="hT_relu")
                    nc.scalar.activation(hT_relu[:], convT[:], AF.Relu, bias=b_l[:])

                    # scores (node-major [128, T])
                    score_ps = ps_tp.tile([128, T], dt.float32, tag="tp")
                    for t in range(T):
                        nc.tensor.matmul(out=score_ps[:, t:t + 1],
                                         lhsT=hT_relu[:, t, :], rhs=p_l[:],
                                         start=True, stop=True)
                    s1 = sm.tile([128, T], dt.float32, tag="s1")
                    nc.vector.tensor_tensor(s1[:], score_ps[:], nmask[:], op=ALU.mult)
                    scoreM = sm.tile([128, T], dt.float32, tag="scoreM")
                    nc.vector.tensor_tensor(scoreM[:], s1[:], ninf_pen[:], op=ALU.add)
                    neg = sm.tile([128, T], dt.float32, tag="neg")
                    nc.vector.tensor_scalar_mul(neg[:], scoreM[:], -1.0)

                    kth_t = sm.tile([128, 2], dt.float32, tag="kth")
                    nc.gpsimd.kth_largest(kth_t[:1, :2], neg[:], T, K_NEG[l],
                                          quantile=Q_KTH[l])
                    v_bc = sm.tile([128, 1], dt.float32, tag="vbc")
                    nc.gpsimd.partition_broadcast(v_bc[:], kth_t[:1, 1:2])
                    keep = sm.tile([128, T], dt.float32, tag="keep")
                    nc.vector.tensor_scalar(out=keep[:], in0=neg[:],
                                            scalar1=v_bc[:], scalar2=None,
                                            op0=ALU.is_le)
                    tanh_s = sm.tile([128, T], dt.float32, tag="tanh_s")
                    nc.scalar.activation(tanh_s[:], scoreM[:], AF.Tanh)
                    gate = sm.tile([128, T], dt.float32, tag="gate")
                    nc.vector.tensor_tensor(gate[:], keep[:], tanh_s[:], op=ALU.mult)
                    ninf_new = sm.tile([128, T], dt.float32, tag="ninf_new")
                    nc.vector.tensor_scalar(out=ninf_new[:], in0=keep[:],
                                            scalar1=NEG_BIG, scalar2=-NEG_BIG,
                                            op0=ALU.mult, op1=ALU.add)

                    # node-major gated h (+ split), penalized copy for max-readout
                    tpa = ps_tp.tile([128, T, 128], dt.float32, tag="tp")
                    for t in range(T):
                        nc.tensor.transpose(out=tpa[:, t, :], in_=hT_relu[:, t, :],
                                            identity=ident[:])
                    h_pool = st1.tile([128, T, 128], dt.float32, tag="h_pool")
                    nc.vector.tensor_tensor(
                        h_pool[:], tpa[:],
                        gate[:, :, None].to_broadcast([128, T, 128]), op=ALU.mult)
                    nh_hi = st2.tile([128, T, 128], dt.float32r, tag="h_hi")
                    nc.vector.tensor_copy(nh_hi[:], h_pool[:])
                    nh_lo = st2.tile([128, T, 128], dt.float32r, tag="h_lo")
                    nc.vector.tensor_tensor(out=nh_lo[:], in0=h_pool[:],
                                            in1=nh_hi[:].bitcast(dt.float32),
                                            op=ALU.subtract)
                    h_pen = st1.tile([128, T, 128], dt.float32, tag="h_pen")
                    nc.vector.tensor_tensor(
                        h_pen[:], h_pool[:],
                        ninf_new[:, :, None].to_broadcast([128, T, 128]), op=ALU.add)

                    if l < 2:
                        tpb = ps_tp.tile([128, T, 128], dt.float32, tag="tp")
                        for t in range(T):
                            nc.tensor.transpose(out=tpb[:, t, :], in_=h_pool[:, t, :],
                                                identity=ident[:])
                        nhT_hi = st2.tile([128, T, 128], dt.float32r, tag="hT_hi")
                        nc.scalar.copy(nhT_hi[:], tpb[:])
                        nhT_lo = st2.tile([128, T, 128], dt.float32r, tag="hT_lo")
                        nc.vector.tensor_tensor(out=nhT_lo[:], in0=tpb[:],
                                                in1=nhT_hi[:].bitcast(dt.float32),
                                                op=ALU.subtract)
                        hT_hi, hT_lo = nhT_hi, nhT_lo

                    tpc = ps_tp.tile([128, T, 128], dt.float32, tag="tp")
                    for t in range(T):
                        nc.tensor.transpose(out=tpc[:, t, :], in_=h_pen[:, t, :],
                                            identity=ident[:])
                    mx = sm.tile([128, 1], dt.float32, tag="mx")
                    nc.vector.tensor_reduce(mx[:], tpc[:],
                                            axis=mybir.AxisListType.XYZW, op=ALU.max)
                    nc.vector.tensor_tensor(maxsum[:, g:g + 1], maxsum[:, g:g + 1],
                                            mx[:], op=ALU.add)

                    msum = ps_tp.tile([128, 1], dt.float32, tag="tp")
                    for t in range(T):
                        nc.tensor.matmul(out=msum[:], lhsT=h_pool[:, t, :],
                                         rhs=ones_col[:],
                                         start=(t == 0), stop=(t == T - 1))
                    mean_t = sm.tile([128, 1], dt.float32, tag="mean_t")
                    nc.vector.tensor_scalar_mul(mean_t[:], msum[:], 1.0 / K_DENOM[l])
                    nc.vector.tensor_tensor(meansum[:, g:g + 1], meansum[:, g:g + 1],
                                            mean_t[:], op=ALU.add)

                    h_hi, h_lo = nh_hi, nh_lo
                    nmask, ninf_pen = keep, ninf_new

            # ---------- MLP head (plain fp32) ----------
            y1_ps = ps_tp.tile([H, GPC], dt.float32, tag="tp")
            nc.tensor.matmul(out=y1_ps[:], lhsT=wl1a[:], rhs=maxsum[:],
                             start=True, stop=False)
            nc.tensor.matmul(out=y1_ps[:], lhsT=wl1b[:], rhs=meansum[:],
                             start=False, stop=True)
            y1T = sm.tile([H, GPC], dt.float32, tag="y1T")
            nc.scalar.activation(y1T[:], y1_ps[:], AF.Relu, bias=bl1c[:])

            y2_ps = ps_tp.tile([64, GPC], dt.float32, tag="tp")
            nc.tensor.matmul(out=y2_ps[:], lhsT=wl2_f[:], rhs=y1T[:],
                             start=True, stop=True)
            y2T = sm.tile([64, GPC], dt.float32, tag="y2T")
            nc.scalar.activation(y2T[:], y2_ps[:], AF.Relu, bias=bl2c[:])

            y3_ps = ps_tp.tile([GPC, 7], dt.float32, tag="tp")
            nc.tensor.matmul(out=y3_ps[:], lhsT=y2T[:], rhs=wl3_f[:],
                             start=True, stop=False)
            nc.tensor.matmul(out=y3_ps[:], lhsT=ones8[:], rhs=bl3r[:],
                             start=False, stop=True)

            # log_softmax over classes (free axis)
            m8 = sm.tile([GPC, 1], dt.float32, tag="m8")
            nc.vector.tensor_reduce(m8[:], y3_ps[:], axis=mybir.AxisListType.XYZW,
                                    op=ALU.max)
            xm = sm.tile([GPC, 7], dt.float32, tag="xm")
            nc.vector.tensor_scalar(out=xm[:], in0=y3_ps[:], scalar1=m8[:],
                                    scalar2=None, op0=ALU.subtract)
            ex = sm.tile([GPC, 7], dt.float32, tag="ex")
            nc.scalar.activation(ex[:], xm[:], AF.Exp)
            se = sm.tile([GPC, 1], dt.float32, tag="se")
            nc.vector.tensor_reduce(se[:], ex[:], axis=mybir.AxisListType.XYZW,
                                    op=ALU.add)
            ls = sm.tile([GPC, 1], dt.float32, tag="ls")
            nc.scalar.activation(ls[:], se[:], AF.Ln)
            outv = sm.tile([GPC, 7], dt.float32, tag="outv")
            nc.vector.tensor_scalar(out=outv[:], in0=xm[:], scalar1=ls[:],
                                    scalar2=None, op0=ALU.subtract)
            nc.sync.dma_start(out_d[:], outv[:])

    nc.compile()
    return nc


_NC = None


def _get_nc():
    global _NC
    if _NC is None:
        _NC = _build()
    return _NC


def _rne12(x):
    """Round fp32 array to fp32r grid (12-bit mantissa, RNE) — bit-exact cast."""
    u = np.ascontiguousarray(x, dtype=np.float32).view(np.uint32)
    lsb = (u >> 12) & 1
    ur = (u + 0x7FF + lsb) & np.uint32(0xFFFFF000)
    return ur.view(np.float32)


def _host_prep(x, edge_index, w1r, b1, w1l, w2r, b2, w2l, w3r, b3, w3l,
               p1w, p2w, w_lin1, b_lin1, w_lin2, b_lin2, w_lin3, b_lin3):
    x = np.asarray(x, dtype=np.float32)
    src = np.asarray(edge_index[0], dtype=np.int64)
    dst = np.asarray(edge_index[1], dtype=np.int64)

    # dense per-graph src->dst adjacency (edge multiplicity counts)
    g_ids = src // NP
    key = g_ids * (NP * NP) + (src % NP) * NP + (dst % NP)
    counts = np.bincount(key, minlength=B * NP * NP)
    A = counts.reshape(B, NP, NP).astype(np.float32)

    # x tiled for SBUF: [128, B*T, 4]; transposed x: [4, B, NP]
    xt = x.reshape(B * T, 128, 4).transpose(1, 0, 2)
    xT = x.reshape(B, NP, 4).transpose(2, 0, 1)
    xt_hi = _rne12(xt)
    xt_lo = _rne12(xt - xt_hi)
    xT_hi = _rne12(xT)
    xT_lo = _rne12(xT - xT_hi)

    def col(v):
        return np.ascontiguousarray(np.asarray(v, np.float32).reshape(-1, 1))

    shared = {
        "w1r": np.asarray(w1r, np.float32), "w1l": np.asarray(w1l, np.float32),
        "w2r": np.asarray(w2r, np.float32), "w2l": np.asarray(w2l, np.float32),
        "w3r": np.asarray(w3r, np.float32), "w3l": np.asarray(w3l, np.float32),
        "b1c": col(b1), "b2c": col(b2), "b3c": col(b3),
        "p1": col(p1w), "p2": col(p2w),
        "wl1": np.asarray(w_lin1, np.float32),
        "bl1c": col(b_lin1),
        "wl2": np.asarray(w_lin2, np.float32),
        "bl2c": col(b_lin2),
        "wl3": np.asarray(w_lin3, np.float32),
        "bl3r": np.ascontiguousarray(np.asarray(b_lin3, np.float32).reshape(1, -1)),
    }

    in_maps = []
    for c in range(NCORES):
        gs = slice(c * GPC, (c + 1) * GPC)
        ts = slice(c * GPC * T, (c + 1) * GPC * T)
        m = dict(shared)
        m["adj"] = np.ascontiguousarray(A[gs].reshape(GPC * NP, NP))
        m["xt_hi"] = np.ascontiguousarray(xt_hi[:, ts, :])
        m["xt_lo"] = np.ascontiguousarray(xt_lo[:, ts, :])
        m["xT_hi"] = np.ascontiguousarray(xT_hi[:, gs, :])
        m["xT_lo"] = np.ascontiguousarray(xT_lo[:, gs, :])
        in_maps.append(m)
    return in_maps


def kernel(**inputs):
    nc = _get_nc()
    in_maps = _host_prep(**inputs)
    res = run_bass_kernel_spmd(nc, in_maps, list(range(NCORES)))
    out = np.concatenate([res.results[c]["out"] for c in range(NCORES)], axis=0)
    return out.astype(np.float32)


def run_traced(**inputs):
    """Like kernel() but with NTFF profiling; returns (out, exec_time_ns)."""
    try:
        import ntff_hook  # noqa: F401  (registers the axon NTFF profile hook)
    except Exception:
        pass
    nc = _get_nc()
    in_maps = _host_prep(**inputs)
    res = run_bass_kernel_spmd(nc, in_maps, list(range(NCORES)), trace=True)
    out = np.concatenate([res.results[c]["out"] for c in range(NCORES)], axis=0)
    return out.astype(np.float32), res.exec_time_ns
